# revision 22
# baseline (speedup 1.0000x reference)
"""2-layer GAT on 8 Trainium2 NeuronCores — bf16 transpose-gather design.

Strategy: dst-shard nodes across cores (6250 each, padded to 6272). Per layer:
each core computes packed bf16 node rows [h | a_src-replicated] (256 bf16 =
512B) for its shard via one PE matmul against a pre-replicated weight block,
AllGathers the rows into a replicated DRAM table, then processes its own dst
nodes in supertiles (S = 128*J nodes): dma_gather(transpose=True) pulls the
512B rows for every incoming edge-slot and lands them CHANNELS-ON-PARTITIONS,
i.e. a [128, 2, S*D] tile whose plane0 is h by channel and plane1 is a_src
replicated per head-block. All attention math is then unit-stride full-width
vector work: e = plane1 + a_dstT (broadcast per node segment), leaky/exp on
ACT, weighted payload and denominator via segmented free-dim reduces. The
aggregated [channels, nodes] tile feeds layer 2's matmul directly as lhsT (no
PE transpose). int16 gather indices limit one gather to 32768 table rows, so
each supertile does two gathers (table halves lo/hi) and sums the partials.
"""

import numpy as np

N = 50000
E = 800000
R = 8
NPC = N // R  # 6250 owned nodes per core
TPC = 49  # tiles of 128 nodes
NPAD = TPC * 128  # 6272 rows per shard
HALF = 4 * NPAD  # 25088 table rows per half
IN_CH = 128
HIDDEN = 32
HEADS = 4
OUT_CH = 64
NEG_SLOPE = 0.2
EL = 256  # table row: 256 bf16 = 512B: [h(<=128) | pad | as_rep(128)]
SENT = 6250  # sentinel row (first pad row of core 0 / core 4) in each half
CAP = 26  # supertile packing: J*(Dl+Dh) <= CAP (bounds SBUF tile sizes)
NEG = -1e30


# ---------------------------------------------------------------- host planner
def _build_plan(edge_index):
    src = np.concatenate([edge_index[0], np.arange(N, dtype=np.int64)]).astype(np.int64)
    dst = np.concatenate([edge_index[1], np.arange(N, dtype=np.int64)]).astype(np.int64)
    lo = src < (N // 2)  # src owned by cores 0-3 -> table half 0

    d_lo = np.bincount(dst[lo], minlength=N)
    d_hi = np.bincount(dst[~lo], minlength=N)

    # per-core permutation: sort desc by degree profile; pads (deg 0) at end
    perms = []
    pos = np.empty(N, dtype=np.int64)
    for c in range(R):
        ids = np.arange(c * NPC, (c + 1) * NPC)
        key = np.maximum(d_lo[ids], d_hi[ids]) * 1000 + d_lo[ids] + d_hi[ids]
        order = np.argsort(-key, kind="stable")
        p = ids[order]
        perms.append(p)
        pos[p] = c * NPAD + np.arange(NPC)

    # shared per-tile max degrees (max across cores so supers are uniform)
    dlo_t = np.zeros(TPC, dtype=np.int64)
    dhi_t = np.zeros(TPC, dtype=np.int64)
    for c in range(R):
        dl = np.concatenate([d_lo[perms[c]], np.zeros(NPAD - NPC, dtype=np.int64)])
        dh = np.concatenate([d_hi[perms[c]], np.zeros(NPAD - NPC, dtype=np.int64)])
        dlo_t = np.maximum(dlo_t, dl.reshape(TPC, 128).max(axis=1))
        dhi_t = np.maximum(dhi_t, dh.reshape(TPC, 128).max(axis=1))
    dlo_t = np.maximum(dlo_t, 1)
    dhi_t = np.maximum(dhi_t, 1)

    supers = []  # (t0, J, Dl, Dh)
    t = 0
    while t < TPC:
        for J in (4, 2, 1):
            if t + J <= TPC:
                Dl = int(dlo_t[t : t + J].max())
                Dh = int(dhi_t[t : t + J].max())
                if J * (Dl + Dh) <= CAP or J == 1:
                    break
        supers.append((t, J, Dl, Dh))
        t += J

    slots = sum(128 * J * (Dl + Dh) for (_, J, Dl, Dh) in supers)
    plan = {"supers": supers, "perms": perms, "pos": pos,
            "inflation": slots * R / (E + N)}

    # per-core wrapped int16 gather indices; same columns reused for L1/L2
    gidx_cores = []
    for c in range(R):
        own = (dst >= c * NPC) & (dst < (c + 1) * NPC)
        s_own = src[own]
        d_own = dst[own]
        half = (~(s_own < (N // 2))).astype(np.int64)
        lpos = pos[d_own] - c * NPAD  # local permuted pos of dst
        key = lpos * 2 + half
        order = np.argsort(key, kind="stable")
        key_s = key[order]
        sp = pos[s_own][order]
        first = np.searchsorted(key_s, key_s)
        rank = np.arange(len(key_s)) - first

        cols = []
        for t0, J, Dl, Dh in supers:
            n0, n1 = t0 * 128, (t0 + J) * 128
            S = n1 - n0
            for h, D, base in ((0, Dl, 0), (1, Dh, HALF)):
                tab = np.full((S, D), SENT, dtype=np.int64)
                sel = (key_s % 2 == h) & (key_s // 2 >= n0) & (key_s // 2 < n1)
                rr = rank[sel]
                assert (rr < D).all(), "rank exceeded tile max degree"
                tab[key_s[sel] // 2 - n0, rr] = sp[sel] - base
                flat = tab.reshape(-1).astype(np.int16)  # flat[n*D+k]
                wrapped = flat.reshape(-1, 16)  # [NI/16, 16]
                cols.append(np.tile(wrapped.T, (8, 1)))
        gidx_cores.append(np.concatenate(cols, axis=1))
    plan["gidx"] = gidx_cores
    plan["W"] = gidx_cores[0].shape[1]
    assert all(g.shape[1] == plan["W"] for g in gidx_cores)
    return plan


# ---------------------------------------------------------------- bass kernel
def _build_bass(plan, phases="ABC"):
    import concourse.bacc as bacc
    import concourse.mybir as mybir
    import concourse.tile as tile
    from concourse.masks import make_identity

    f32 = mybir.dt.float32
    bf16 = mybir.dt.bfloat16
    i16 = mybir.dt.int16
    AX = mybir.AxisListType.X
    OP = mybir.AluOpType
    AF = mybir.ActivationFunctionType

    supers = plan["supers"]
    W = plan["W"]

    nc = bacc.Bacc(
        "TRN2",
        target_bir_lowering=False,
        debug=False,
        num_devices=R,
        num_swdge_queues=4,
        dynamic_dma_scratch_size=32768,
    )
    xT_in = nc.dram_tensor("xT", [128, NPAD], f32, kind="ExternalInput")
    gidx_in = nc.dram_tensor("gidx", [128, W], i16, kind="ExternalInput")
    w1e_in = nc.dram_tensor("w1e", [128, EL], f32, kind="ExternalInput")
    wd1_in = nc.dram_tensor("wd1r", [128, 128], f32, kind="ExternalInput")
    w2e_in = nc.dram_tensor("w2e", [128, EL], bf16, kind="ExternalInput")
    wd2_in = nc.dram_tensor("wd2r", [128, 128], bf16, kind="ExternalInput")
    b1_in = nc.dram_tensor("b1c", [128, 1], f32, kind="ExternalInput")
    b2_in = nc.dram_tensor("b2c", [128, 1], f32, kind="ExternalInput")
    out_d = nc.dram_tensor("out", [NPAD, 64], f32, kind="ExternalOutput")

    with tile.TileContext(nc) as tc:
        with (
            tc.tile_pool(name="const", bufs=1) as cp,
            tc.tile_pool(name="work", bufs=2) as wp,
            tc.tile_pool(name="small", bufs=3) as sp,
            tc.tile_pool(name="gath", bufs=3) as gp,
            tc.tile_pool(name="psum", bufs=2, space="PSUM") as pp,
            tc.tile_pool(name="dram", bufs=1, space="DRAM") as dp,
        ):
            shard1 = dp.tile([NPAD, EL], bf16)
            table1 = dp.tile([R * NPAD, EL], bf16, addr_space="Shared")
            shard2 = dp.tile([NPAD, EL], bf16)
            table2 = dp.tile([R * NPAD, EL], bf16, addr_space="Shared")

            w1e = cp.tile([128, EL], f32)
            nc.sync.dma_start(out=w1e[:], in_=w1e_in[:])
            wd1 = cp.tile([128, 128], f32)
            nc.sync.dma_start(out=wd1[:], in_=wd1_in[:])
            w2e = cp.tile([128, EL], bf16)
            nc.sync.dma_start(out=w2e[:], in_=w2e_in[:])
            wd2 = cp.tile([128, 128], bf16)
            nc.sync.dma_start(out=wd2[:], in_=wd2_in[:])
            b1c = cp.tile([128, 1], f32)
            nc.sync.dma_start(out=b1c[:], in_=b1_in[:])
            b2c = cp.tile([128, 1], f32)
            nc.sync.dma_start(out=b2c[:], in_=b2_in[:])
            ident = cp.tile([128, 128], f32)
            make_identity(nc, ident[:])
            idxall = cp.tile([128, W], i16)
            nc.sync.dma_start(out=idxall[:], in_=gidx_in[:])
            negbf = cp.tile([128, 128], bf16)
            nc.gpsimd.memset(negbf[:], NEG)
            adT1 = cp.tile([128, NPAD], bf16)
            adT2 = cp.tile([128, NPAD], bf16)

            # ---------------- phase A: packed rows + dst-coef transpose
            for t in range(TPC):
                xt = wp.tile([128, 128], f32, tag="xt")
                nc.sync.dma_start(out=xt[:], in_=xT_in[:, t * 128 : (t + 1) * 128])
                psA = pp.tile([128, EL], f32, tag="mm256")
                nc.tensor.matmul(psA[:], lhsT=xt[:], rhs=w1e[:], start=True, stop=True)
                hbf = wp.tile([128, EL], bf16, tag="hbf")
                nc.scalar.copy(out=hbf[:], in_=psA[:])
                nc.sync.dma_start(
                    out=shard1[t * 128 : (t + 1) * 128, :], in_=hbf[:]
                )
                psD = pp.tile([128, 128], f32, tag="mm128")
                nc.tensor.matmul(psD[:], lhsT=wd1[:], rhs=xt[:], start=True, stop=True)
                nc.scalar.copy(out=adT1[:, t * 128 : (t + 1) * 128], in_=psD[:])
            # pad rows: as_rep = -1e30 so padded slots vanish in the softmax
            nc.sync.dma_start(
                out=shard1[NPC:NPAD, 128:EL], in_=negbf[0 : NPAD - NPC, :]
            )

            nc.gpsimd.collective_compute(
                "AllGather",
                mybir.AluOpType.bypass,
                replica_groups=[list(range(R))],
                ins=[shard1.opt()],
                outs=[table1.opt()],
            )

            import os as _os
            _agd = int(_os.environ.get("AG_DELAY", "0"))
            if _agd:
                idxsave = cp.tile([128, 8], i16)
                nc.vector.tensor_copy(out=idxsave[:], in_=idxall[:, 0:8])
                dummy = cp.tile([128, 4096], bf16)
                nc.gpsimd.memset(dummy[:], 0.0)
                # anchor: read table1 so the chain starts after the AllGather
                nc.sync.dma_start(out=dummy[:, 0:256], in_=table1[0:128, :])
                for _i in range(_agd):
                    nc.scalar.copy(out=dummy[:], in_=dummy[:])
                nc.gpsimd.memset(dummy[:, 0:8], 0.0)
                # value-neutral rewrite of idx columns, ordered after the delay
                # chain via dummy; all gathers (idxall readers) now wait for it
                nc.vector.tensor_tensor(
                    out=idxall[:, 0:8],
                    in0=idxsave[:],
                    in1=dummy[:, 0:8].bitcast(i16),
                    op=OP.bitwise_or,
                )

            if phases == "A":
                dbgb = wp.tile([128, 64], bf16, tag="dbgb")
                nc.sync.dma_start(out=dbgb[:], in_=table1[0:128, 0:64])
                dbg = wp.tile([128, 64], f32, tag="dbg")
                nc.scalar.copy(out=dbg[:], in_=dbgb[:])
                nc.sync.dma_start(out=out_d[0:128, :], in_=dbg[:])

            colmap = {}
            _c = 0
            for _si, (_t0, _J, _Dl, _Dh) in enumerate(supers):
                colmap[_t0] = _c
                _c += 128 * _J * (_Dl + _Dh) // 16
            qrr = [0]

            def gather_pair(table, t0, J, Dl, Dh):
                outs = []
                S = 128 * J
                c0 = colmap[t0]
                for D, base0, base1 in ((Dl, 0, HALF), (Dh, HALF, R * NPAD)):
                    NI = S * D
                    g = gp.tile([128, 2 * NI], bf16, tag=f"g{base0 != 0}")
                    nc.gpsimd.dma_gather(
                        g[:].rearrange("p (q i) -> p q i", q=2),
                        table[base0:base1, :],
                        idxall[:, c0 : c0 + NI // 16],
                        NI,
                        NI,
                        EL,
                        transpose=True,
                        single_packet=False,
                        queue_num=qrr[0] % 4,
                    )
                    qrr[0] += 1
                    c0 += NI // 16
                    outs.append((g, NI, D))
                return outs

            def attn_half(g, NI, D, S, adT, n0, hv):
                """One table-half: returns (raw [128,S] f32, den [128,S] f32).
                scr is computed in-place over the gathered payload plane."""
                plane0 = g[:, 0:NI]
                plane1 = g[:, NI : 2 * NI]
                et = wp.tile([128, NI], bf16, tag="et")
                nc.vector.tensor_tensor(
                    out=et[:].rearrange("p (s d) -> p s d", d=D),
                    in0=plane1.rearrange("p (s d) -> p s d", d=D),
                    in1=adT[:, n0 : n0 + S].unsqueeze(2).to_broadcast([128, S, D]),
                    op=OP.add,
                )
                esc = wp.tile([128, NI], bf16, tag="esc")
                nc.scalar.mul(esc[:], et[:], NEG_SLOPE)
                nc.vector.tensor_tensor(et[:], et[:], esc[:], op=OP.max)
                ext = wp.tile([128, NI], bf16, tag="ext")
                nc.scalar.activation(ext[:], et[:], AF.Exp)
                nc.vector.tensor_tensor(plane0, plane0, ext[:], op=OP.mult)
                raw = sp.tile([128, S], f32, tag=f"raw{hv}")
                nc.vector.reduce_sum(
                    out=raw[:], in_=plane0.rearrange("p (s d) -> p s d", d=D), axis=AX
                )
                den = sp.tile([128, S], f32, tag=f"den{hv}")
                nc.vector.reduce_sum(
                    out=den[:], in_=ext[:].rearrange("p (s d) -> p s d", d=D), axis=AX
                )
                return raw, den

            # ---------------- phase B: layer-1 aggregation + layer-2 features
            from collections import deque

            def run_phase(table, adT, bias, body, PF=2):
                pending = deque()
                for t0, J, Dl, Dh in supers:
                    pending.append((t0, J, gather_pair(table, t0, J, Dl, Dh)))
                    if len(pending) > PF:
                        body(*pending.popleft())
                while pending:
                    body(*pending.popleft())

            def agg_tail(adT, t0, J, pair, bias):
                S = 128 * J
                (glo, NIl, Dl), (ghi, NIh, Dh) = pair
                raw_l, den_l = attn_half(glo, NIl, Dl, S, adT, t0 * 128, 0)
                raw_h, den_h = attn_half(ghi, NIh, Dh, S, adT, t0 * 128, 1)
                nc.vector.tensor_tensor(raw_l[:], raw_l[:], raw_h[:], op=OP.add)
                nc.vector.tensor_tensor(den_l[:], den_l[:], den_h[:], op=OP.add)
                nc.vector.tensor_scalar_add(den_l[:], den_l[:], 1e-16)
                rden = sp.tile([128, S], f32, tag="rden")
                nc.vector.reciprocal_approx_fast(rden[:], den_l[:])
                nc.vector.tensor_tensor(raw_l[:], raw_l[:], rden[:], op=OP.mult)
                z = sp.tile([128, S], f32, tag="z")
                nc.scalar.activation(z[:], raw_l[:], AF.Identity, bias=bias[:, :1])
                return z

            def body_B(t0, J, pair):
                S = 128 * J
                z = agg_tail(adT1, t0, J, pair, b1c)
                if "R" in phases:
                    for j in range(J):
                        t = t0 + j
                        psR = pp.tile([128, 128], f32, tag="psT")
                        nc.tensor.transpose(
                            psR[:], z[:, j * 128 : (j + 1) * 128], ident[:]
                        )
                        oR = wp.tile([128, 64], f32, tag="o2")
                        nc.scalar.copy(out=oR[:], in_=psR[:, 0:64])
                        nc.sync.dma_start(
                            out=out_d[t * 128 : (t + 1) * 128, :], in_=oR[:]
                        )
                    return
                # ELU(z) = relu(z) + exp(-relu(-z)) - 1
                mt = sp.tile([128, S], f32, tag="mt")
                nc.scalar.activation(mt[:], z[:], AF.Relu, scale=-1.0)
                nc.scalar.activation(mt[:], mt[:], AF.Exp, scale=-1.0)
                rt = sp.tile([128, S], f32, tag="rt")
                nc.scalar.activation(rt[:], z[:], AF.Relu)
                nc.vector.tensor_tensor(rt[:], rt[:], mt[:], op=OP.add)
                elut = sp.tile([128, S], bf16, tag="elut")
                nc.vector.tensor_scalar_add(elut[:], rt[:], -1.0)
                for j in range(J):
                    t = t0 + j
                    lhs = elut[:, j * 128 : (j + 1) * 128]
                    ps2 = pp.tile([128, EL], f32, tag="mm256")
                    nc.tensor.matmul(ps2[:], lhsT=lhs, rhs=w2e[:], start=True, stop=True)
                    h2bf = wp.tile([128, EL], bf16, tag="h2bf")
                    nc.scalar.copy(out=h2bf[:], in_=ps2[:])
                    nc.sync.dma_start(
                        out=shard2[t * 128 : (t + 1) * 128, :], in_=h2bf[:]
                    )
                    psD2 = pp.tile([128, 128], f32, tag="mm128")
                    nc.tensor.matmul(psD2[:], lhsT=wd2[:], rhs=lhs, start=True, stop=True)
                    nc.scalar.copy(out=adT2[:, t * 128 : (t + 1) * 128], in_=psD2[:])

            if "B" in phases:
                run_phase(table1, adT1, b1c, body_B)

            if "C" in phases:
                nc.sync.dma_start(
                    out=shard2[NPC:NPAD, 128:EL], in_=negbf[0 : NPAD - NPC, :]
                )
                if _agd:
                    # pre-AG2 fence: anchor on shard2 (waits all writers), pad
                    # time, then a harmless write AG2 must wait for
                    nc.sync.dma_start(out=dummy[:, 0:256], in_=shard2[0:128, :])
                    for _i in range(_agd // 2):
                        nc.scalar.copy(out=dummy[:], in_=dummy[:])
                    nc.gpsimd.memset(dummy[:, 0:8], 0.0)
                    nc.sync.dma_start(
                        out=shard2[NPC : NPC + 1, 0:8], in_=dummy[0:1, 0:8]
                    )
                nc.gpsimd.collective_compute(
                    "AllGather",
                    mybir.AluOpType.bypass,
                    replica_groups=[list(range(R))],
                    ins=[shard2.opt()],
                    outs=[table2.opt()],
                )
                if _agd:
                    nc.sync.dma_start(out=dummy[:, 0:256], in_=table2[0:128, :])
                    for _i in range(_agd):
                        nc.scalar.copy(out=dummy[:], in_=dummy[:])
                    nc.gpsimd.memset(dummy[:, 0:8], 0.0)
                    nc.vector.tensor_tensor(
                        out=idxall[:, 0:8],
                        in0=idxsave[:],
                        in1=dummy[:, 0:8].bitcast(i16),
                        op=OP.bitwise_or,
                    )

            # ---------------- phase C: layer-2 aggregation + output
            def body_C(t0, J, pair):
                z2 = agg_tail(adT2, t0, J, pair, b2c)
                for j in range(J):
                    t = t0 + j
                    psT = pp.tile([128, 128], f32, tag="psT")
                    nc.tensor.transpose(
                        psT[:], z2[:, j * 128 : (j + 1) * 128], ident[:]
                    )
                    o2 = wp.tile([128, 64], f32, tag="o2")
                    nc.scalar.copy(out=o2[:], in_=psT[:, 0:64])
                    nc.sync.dma_start(
                        out=out_d[t * 128 : (t + 1) * 128, :], in_=o2[:]
                    )

            if "C" in phases:
                run_phase(table2, adT2, b2c, body_C)

    nc.finalize()
    return nc


# ---------------------------------------------------------------- entry point
_cache = {}


def kernel(x, edge_index, W1, att_src1, att_dst1, b1, W2, att_src2, att_dst2, b2):
    import ml_dtypes
    from concourse.bass_utils import run_bass_kernel_spmd

    x = np.asarray(x, dtype=np.float32)
    edge_index = np.asarray(edge_index, dtype=np.int64)
    W1 = np.asarray(W1, dtype=np.float32)
    W2 = np.asarray(W2, dtype=np.float32)
    att_src1 = np.asarray(att_src1, dtype=np.float32)
    att_dst1 = np.asarray(att_dst1, dtype=np.float32)
    att_src2 = np.asarray(att_src2, dtype=np.float32)
    att_dst2 = np.asarray(att_dst2, dtype=np.float32)
    b1 = np.asarray(b1, dtype=np.float32)
    b2 = np.asarray(b2, dtype=np.float32)

    import os

    phases = os.environ.get("KERNEL_PHASES", "ABC")
    key = (hash(edge_index.tobytes()), phases)
    if "plan" not in _cache or _cache.get("key") != key:
        _cache["plan"] = _build_plan(edge_index)
        _cache["nc"] = _build_bass(_cache["plan"], phases)
        _cache["key"] = key
    plan = _cache["plan"]
    nc = _cache["nc"]

    # weight packing: as = x @ (W1 . att_src) etc., head-replicated out blocks
    W1r = W1.reshape(IN_CH, HEADS, HIDDEN)
    Ws1 = np.einsum("khc,hc->kh", W1r, att_src1)  # [128, 4]
    Wd1 = np.einsum("khc,hc->kh", W1r, att_dst1)
    rep = np.repeat(np.arange(HEADS), HIDDEN)  # head index per partition
    w1e = np.concatenate([W1, Ws1[:, rep]], axis=1).astype(np.float32)  # [128, 256]
    wd1r = Wd1[:, rep].astype(np.float32)  # [128, 128]
    Ws2 = W2 @ att_src2[0]  # [128]
    Wd2 = W2 @ att_dst2[0]
    w2e = np.concatenate(
        [W2, np.zeros((128, 128 - OUT_CH), np.float32), np.tile(Ws2[:, None], 128)],
        axis=1,
    ).astype(ml_dtypes.bfloat16)  # [128, 256]
    wd2r = np.tile(Wd2[:, None], 128).astype(ml_dtypes.bfloat16)
    b2c = np.zeros((128, 1), np.float32)
    b2c[:OUT_CH, 0] = b2

    in_maps = []
    for c in range(R):
        xp = np.zeros((NPAD, IN_CH), dtype=np.float32)
        xp[:NPC] = x[plan["perms"][c]]
        in_maps.append(
            {
                "xT": np.ascontiguousarray(xp.T),
                "gidx": plan["gidx"][c],
                "w1e": w1e,
                "wd1r": wd1r,
                "w2e": w2e,
                "wd2r": wd2r,
                "b1c": b1.reshape(128, 1).astype(np.float32),
                "b2c": b2c,
            }
        )

    res = run_bass_kernel_spmd(nc, in_maps, core_ids=list(range(R)))
    _cache["last_res"] = res
    out = np.empty((N, OUT_CH), dtype=np.float32)
    for c in range(R):
        out[plan["perms"][c]] = res.results[c]["out"][:NPC]
    return out
